# revision 1
# baseline (speedup 1.0000x reference)
"""Causal self-attention with RoPE on 8 Trainium2 NeuronCores.

Sharding: data-parallel over batch (B=8 -> 1 batch element per core, no
collectives). Per core, everything is computed in transposed layouts so no
on-device transposes are needed:

  - V first: V = x @ Wv^T -> V_aug tiles (V with a ones column and the
    padding mask folded in).
  - Then 8 rounds, one (q, k) head-pair chunk each: Q^T/K^T chunk matmuls
    (W stationary, x^T moving), RoPE via a signed-permutation matmul
    (rot(q) * sin == Perm @ (q * sin) since sin repeats with period 32),
    and the two heads of the PREVIOUS round's chunk: S^T strips per
    key-chunk (causal blocks only, diagonal masked additively in PSUM),
    exp on ACT, O^T/l accumulated in PSUM via lhsT = V_aug, then a fast
    normalize chain off the critical path.
  - Out projection with W_out^T prefetched, y written directly in [t, c].

This interleaving keeps PE busy while ACT does the exp work. All matmuls
run in bf16 (fp32 accumulation in PSUM).
"""

import contextlib

import ml_dtypes
import numpy as np

import concourse.bass as bass  # noqa: F401
import concourse.mybir as mybir
import concourse.tile as tile
from concourse import bacc
from concourse.bass_utils import run_bass_kernel_spmd

P = 128
T = 1024
DM = 1024
H = 16
HD = 64
NT = T // P
NCC = DM // P
F32 = mybir.dt.float32
BF16 = mybir.dt.bfloat16
AOP = mybir.AluOpType
AF = mybir.ActivationFunctionType
NEG = -1.0e30
SCALE = 0.125
SEGB = 512        # matmul free-dim cap (PSUM bank)

LAST_RESULTS = None
_NC_CACHE = None


def _segs(lo, hi, bound=SEGB):
    out = []
    a = lo
    while a < hi:
        b = min(hi, (a // bound + 1) * bound)
        out.append((a, b))
        a = b
    return out


def _build(loop_n=1):
    nc = bacc.Bacc("TRN2", target_bir_lowering=False, debug=False, num_devices=8)
    xT = nc.dram_tensor("xT", [DM, T], BF16, kind="ExternalInput")
    wqk = nc.dram_tensor("wqk", [16, NCC, P, P], BF16, kind="ExternalInput")
    wv = nc.dram_tensor("wv", [2, NCC, P, 512], BF16, kind="ExternalInput")
    wo = nc.dram_tensor("wo", [DM, DM], BF16, kind="ExternalInput")
    cosd = nc.dram_tensor("cosd", [P, T], F32, kind="ExternalInput")
    sind = nc.dram_tensor("sind", [P, T], F32, kind="ExternalInput")
    permd = nc.dram_tensor("permd", [P, P], BF16, kind="ExternalInput")
    mdiagd = nc.dram_tensor("mdiagd", [P, P], F32, kind="ExternalInput")
    maskd = nc.dram_tensor("maskd", [P, NT], F32, kind="ExternalInput")
    y = nc.dram_tensor("y", [T, DM], F32, kind="ExternalOutput")

    with tile.TileContext(nc) as tc:
        with (
            tc.tile_pool(name="constp", bufs=1) as constp,
            tc.tile_pool(name="persist", bufs=1) as persist,
        ):
            cos_sb = constp.tile([P, T], F32, name="cos_sb")
            sin_sb = constp.tile([P, T], F32, name="sin_sb")
            perm_sb = constp.tile([P, P], BF16, name="perm_sb")
            mdiag_sb = constp.tile([P, P], F32, name="mdiag_sb")
            maskv_sb = constp.tile([P, NT], F32, name="maskv_sb")

            qt = persist.tile([P, NCC, T], BF16, name="qt")
            kt = persist.tile([P, NCC, T], BF16, name="kt")
            vaug = persist.tile([P, NT, H, HD + 1], BF16, name="vaug")
            at = persist.tile([P, NCC, T], BF16, name="at")

            loop_ctx = tc.For_i(0, loop_n, 1) if loop_n > 1 \
                else contextlib.nullcontext()
            with loop_ctx:
                _emit_body(
                    nc, tc, xT, wqk, wv, wo, y, cosd, sind, permd, mdiagd,
                    maskd, cos_sb, sin_sb, perm_sb, mdiag_sb, maskv_sb,
                    qt, kt, vaug, at,
                )

    nc.compile()
    return nc


def _emit_body(nc, tc, xT, wqk, wv, wo, y, cosd, sind, permd, mdiagd, maskd,
               cos_sb, sin_sb, perm_sb, mdiag_sb, maskv_sb, qt, kt, vaug, at):
    with (
        tc.tile_pool(name="xtp", bufs=1) as xtp,
        tc.tile_pool(name="wqkp", bufs=3) as wqkp,
        tc.tile_pool(name="ropet", bufs=4) as ropet,
        tc.tile_pool(name="wop", bufs=2) as wop,
        tc.tile_pool(name="ptp", bufs=3) as ptp,
        tc.tile_pool(name="ocp", bufs=2) as ocp,
        tc.tile_pool(name="rcp", bufs=2) as rcp,
        tc.tile_pool(name="bcp", bufs=2) as bcp,
        tc.tile_pool(name="tnp", bufs=2) as tnp,
        tc.tile_pool(name="wvp", bufs=2) as wvp,
    ):
        xt = xtp.tile([P, NCC, T], BF16, name="xt")
        nc.sync.dma_start(xt[:, 0, :], xT[0:P, :])
        wt_pre = {}
        for ocg in (0, 8):  # chunk-0 Q and K weights, ahead of the bulk loads
            wt = wqkp.tile([P, NCC, P], BF16, name="wt", tag="wt")
            nc.sync.dma_start(wt[:], wqk[ocg].rearrange("c p n -> p c n"))
            wt_pre[ocg] = wt
        nc.sync.dma_start(xt[:, 1, :], xT[P:2 * P, :])
        nc.sync.dma_start(cos_sb[:], cosd[:])
        nc.sync.dma_start(sin_sb[:], sind[:])
        for cc in range(2, NCC):
            nc.sync.dma_start(xt[:, cc, :], xT[cc * P:(cc + 1) * P, :])
        nc.sync.dma_start(maskv_sb[:], maskd[:])
        nc.sync.dma_start(perm_sb[:], permd[:])
        nc.sync.dma_start(mdiag_sb[:], mdiagd[:])

        # ------------- overlapped QK+RoPE, V, and attention -------------
        wo_half = [None, None]
        with (
            tc.tile_pool(name="ps_big", bufs=3, space="PSUM") as ps_big,
            tc.tile_pool(name="ps_o", bufs=1, space="PSUM") as ps_o,
        ):

            def emit_qk_chunk(ocg):
                """Project o-chunk ocg (0..7 = Q chunks, 8..15 = K chunks)."""
                if ocg in wt_pre:
                    wt = wt_pre.pop(ocg)
                else:
                    wt = wqkp.tile([P, NCC, P], BF16, name="wt", tag="wt")
                    nc.sync.dma_start(wt[:], wqk[ocg].rearrange("c p n -> p c n"))
                acc = ps_big.tile([P, T], F32, name="acc", tag="big")
                for cc in range(NCC):
                    for lo, hi in _segs(0, T):
                        nc.tensor.matmul(
                            acc[:, lo:hi],
                            lhsT=wt[:, cc, :],
                            rhs=xt[:, cc, lo:hi],
                            start=(cc == 0),
                            stop=(cc == NCC - 1),
                        )
                return acc

            def emit_rope(acc, dst, oc):
                for lo, hi in _segs(0, T):
                    w = hi - lo
                    tsin = ropet.tile([P, SEGB], BF16, name="tsin", tag="tsin")
                    tcos = ropet.tile([P, SEGB], F32, name="tcos", tag="tcos")
                    nc.vector.tensor_tensor(
                        tsin[:, :w], acc[:, lo:hi], sin_sb[:, lo:hi], AOP.mult
                    )
                    nc.vector.tensor_tensor(
                        tcos[:, :w], acc[:, lo:hi], cos_sb[:, lo:hi], AOP.mult
                    )
                    rp = ps_big.tile([P, T], F32, name="rp", tag="big")
                    nc.tensor.matmul(
                        rp[:, lo:hi],
                        lhsT=perm_sb[:],
                        rhs=tsin[:, :w],
                        start=True,
                        stop=True,
                    )
                    nc.vector.tensor_tensor(
                        dst[:, oc, lo:hi], rp[:, lo:hi], tcos[:, :w], AOP.add
                    )

            def emit_head(h):
                h2, hp = divmod(h, 2)
                hoff = hp * HD
                po = ps_o.tile([P, T], F32, name="po")

                def emit_s(jc):
                    i0 = jc * P
                    pss = ps_big.tile([P, T], F32, name="pss", tag="big")
                    for lo, hi in _segs(0, T - i0):
                        nc.tensor.matmul(
                            pss[:, lo:hi],
                            lhsT=kt[hoff:hoff + HD, h2, jc * P:(jc + 1) * P],
                            rhs=qt[hoff:hoff + HD, h2, i0 + lo:i0 + hi],
                            start=True,
                            stop=True,
                        )
                    nc.vector.tensor_tensor(
                        pss[:, 0:P], pss[:, 0:P], mdiag_sb[:], AOP.add
                    )
                    pt = ptp.tile([P, T], BF16, name="pt")
                    nc.scalar.activation(
                        pt[:, 0:T - i0], pss[:, 0:T - i0], AF.Exp, scale=SCALE
                    )
                    return pt

                def emit_pv(jc, pt):
                    i0 = jc * P
                    for glo, ghi in _segs(i0, T):
                        nc.tensor.matmul(
                            po[0:HD + 1, glo:ghi],
                            lhsT=vaug[:, jc, h, :],
                            rhs=pt[:, glo - i0:ghi - i0],
                            start=(jc == 0),
                            stop=(jc == NT - 1),
                        )

                pts = [None] * NT
                pts[0] = emit_s(0)
                for jc in range(NT):
                    if jc + 1 < NT:
                        pts[jc + 1] = emit_s(jc + 1)
                    emit_pv(jc, pts[jc])

                # single DVE copy frees `po`; normalize runs off critical path
                oc_sb = ocp.tile([HD + 1, T], F32, name="oc_sb")
                nc.vector.tensor_copy(out=oc_sb[:], in_=po[0:HD + 1, :])
                l128 = rcp.tile([P, NT], F32, name="l128", tag="l128")
                nc.sync.dma_start(l128[:], oc_sb[HD:HD + 1, :])
                nc.vector.reciprocal(l128[:], l128[:])
                rc0 = rcp.tile([1, T], F32, name="rc0", tag="rc0")
                nc.sync.dma_start(rc0[:], l128[:])
                bc = bcp.tile([HD, T], F32, name="bc")
                nc.gpsimd.partition_broadcast(bc[:], rc0[:])
                if hoff == 0:
                    nc.vector.tensor_tensor(
                        at[0:HD, h2, :], oc_sb[0:HD, :], bc[:], AOP.mult
                    )
                else:
                    tn = tnp.tile([HD, T], BF16, name="tn")
                    nc.vector.tensor_tensor(
                        tn[:], oc_sb[0:HD, :], bc[:], AOP.mult
                    )
                    nc.sync.dma_start(at[hoff:hoff + HD, h2, :], tn[:])

            def emit_v():
                for vb in range(2):
                    wvt = wvp.tile([P, NCC, 512], BF16, name="wvt")
                    nc.sync.dma_start(wvt[:], wv[vb].rearrange("c p n -> p c n"))
                    for tcix in range(NT):
                        pvt = ps_big.tile([P, T], F32, name="pvt", tag="big")
                        pv = pvt[:, 0:512]
                        for cc in range(NCC):
                            nc.tensor.matmul(
                                pv,
                                lhsT=xt[:, cc, tcix * P:(tcix + 1) * P],
                                rhs=wvt[:, cc, :],
                                start=(cc == 0),
                                stop=(cc == NCC - 1),
                            )
                        dstv = vaug[:, tcix, vb * 8:(vb + 1) * 8, 0:HD]
                        nc.scalar.activation(
                            dstv,
                            pv.rearrange("p (h d) -> p h d", d=HD),
                            AF.Copy,
                            scale=maskv_sb[:, tcix:tcix + 1],
                        )
                        ones_col = vaug[:, tcix, vb * 8:(vb + 1) * 8, HD:HD + 1]
                        nc.vector.tensor_copy(
                            out=ones_col,
                            in_=maskv_sb[:, tcix:tcix + 1, None].to_broadcast(
                                [P, 8, 1]
                            ),
                        )

            prev = None
            for c in range(NCC):
                qa = emit_qk_chunk(c)        # Q chunk c (heads 2c, 2c+1)
                ka = emit_qk_chunk(8 + c)    # K chunk c
                emit_rope(qa, qt, c)
                if c == 1:
                    emit_v()
                if c == 6:
                    wo_half[0] = wop.tile(
                        [P, NCC, 512], BF16, name="wo_sb", tag="wo"
                    )
                    nc.sync.dma_start(
                        wo_half[0][:],
                        wo[:, 0:512].rearrange("(c p) n -> p c n", p=P),
                    )
                if prev is not None:
                    emit_head(2 * prev + 1)
                    emit_head(2 * prev)
                emit_rope(ka, kt, c)
                prev = c
            emit_head(2 * prev + 1)
            emit_head(2 * prev)

        # ------------- out projection -------------
        with (
            tc.tile_pool(name="youtp", bufs=2) as youtp,
            tc.tile_pool(name="ps_y", bufs=2, space="PSUM") as ps_y,
        ):
            wo_half[1] = wop.tile([P, NCC, 512], BF16, name="wo_sb", tag="wo")
            nc.sync.dma_start(
                wo_half[1][:], wo[:, 512:1024].rearrange("(c p) n -> p c n", p=P)
            )
            def emit_y_group(hf, tcix, ccs, py):
                for cc in ccs:
                    nc.tensor.matmul(
                        py[:],
                        lhsT=at[:, cc, tcix * P:(tcix + 1) * P],
                        rhs=wo_half[hf][:, cc, :],
                        start=(cc == 0),
                        stop=(cc == NCC - 1),
                    )

            def finish_y_group(hf, tcix, py):
                ysb = youtp.tile([P, 512], F32, name="ysb")
                nc.scalar.copy(ysb[:], py[:])
                nc.sync.dma_start(
                    y[tcix * P:(tcix + 1) * P, hf * 512:(hf + 1) * 512],
                    ysb[:],
                )

            # first two groups: run cc 0..6 for both before either cc=7, so
            # PE has work while the last heads' normalize chain finishes
            py0 = ps_y.tile([P, 512], F32, name="py", tag="py")
            emit_y_group(0, 0, range(NCC - 1), py0)
            py1 = ps_y.tile([P, 512], F32, name="py", tag="py")
            emit_y_group(0, 1, range(NCC - 1), py1)
            emit_y_group(0, 0, [NCC - 1], py0)
            finish_y_group(0, 0, py0)
            emit_y_group(0, 1, [NCC - 1], py1)
            finish_y_group(0, 1, py1)
            for hf in range(2):
                for tcix in range(2 if hf == 0 else 0, NT):
                    py = ps_y.tile([P, 512], F32, name="py", tag="py")
                    emit_y_group(hf, tcix, range(NCC), py)
                    finish_y_group(hf, tcix, py)


def _host_consts():
    half = HD // 2
    inv = (1.0 / 10000.0 ** (np.arange(half, dtype=np.float32) / np.float32(half)))
    pos = np.arange(T, dtype=np.float32)
    fr = np.outer(pos, inv)
    emb = np.concatenate([fr, fr], axis=-1)
    cos = np.cos(emb).astype(np.float32)
    sin = np.sin(emb).astype(np.float32)
    cosd = np.ascontiguousarray(np.tile(cos.T, (2, 1)))
    sind = np.ascontiguousarray(np.tile(sin.T, (2, 1)))

    p64 = np.zeros((HD, HD), dtype=np.float32)
    for i in range(half):
        p64[i, i + half] = -1.0
        p64[i + half, i] = 1.0
    perm128 = np.zeros((P, P), dtype=np.float32)
    perm128[:HD, :HD] = p64
    perm128[HD:, HD:] = p64
    permd = np.ascontiguousarray(perm128.T.astype(ml_dtypes.bfloat16))

    jj = np.arange(P)[:, None]
    ii = np.arange(P)[None, :]
    mdiag = np.where(jj > ii, np.float32(NEG), np.float32(0.0)).astype(np.float32)
    return cosd, sind, permd, mdiag


def make_in_maps(x, attention_mask, W_qkv, W_out):
    bf = ml_dtypes.bfloat16
    x = np.asarray(x, dtype=np.float32)
    mask = np.asarray(attention_mask)
    Wt = np.asarray(W_qkv, dtype=np.float32).T
    wqk_t = np.ascontiguousarray(
        Wt[:, :2 * DM].reshape(NCC, P, 16, P).transpose(2, 0, 1, 3).astype(bf)
    )
    wv_t = np.ascontiguousarray(
        Wt[:, 2 * DM:].reshape(NCC, P, 2, 512).transpose(2, 0, 1, 3).astype(bf)
    )
    wo_t = np.ascontiguousarray(np.asarray(W_out, dtype=np.float32).T.astype(bf))
    cosd, sind, permd, mdiag = _host_consts()
    in_maps = []
    for b in range(x.shape[0]):
        in_maps.append({
            "xT": np.ascontiguousarray(x[b].T.astype(bf)),
            "wqk": wqk_t,
            "wv": wv_t,
            "wo": wo_t,
            "cosd": cosd,
            "sind": sind,
            "permd": permd,
            "mdiagd": mdiag,
            "maskd": np.ascontiguousarray(
                mask[b].astype(np.float32).reshape(NT, P).T
            ),
        })
    return in_maps


def kernel(x, attention_mask, W_qkv, W_out):
    global LAST_RESULTS, _NC_CACHE
    if _NC_CACHE is None:
        _NC_CACHE = _build()
    nc = _NC_CACHE
    in_maps = make_in_maps(x, attention_mask, W_qkv, W_out)
    res = run_bass_kernel_spmd(nc, in_maps, list(range(8)))
    LAST_RESULTS = res
    out = np.stack([res.results[b]["y"] for b in range(8)], axis=0)
    return np.ascontiguousarray(out.astype(np.float32))

